# revision 46
# baseline (speedup 1.0000x reference)
"""Trainium2 Bass kernel for GQA attention (B=2,T=2048,D=2048,H=16,G=4,K=128)
with QK RMS-norm, RoPE, segment-aware causal masking, sigmoid gating, o_proj.

Sharding: 8 cores = (batch b, kv-group g); core c -> b=c//4, g=c%4.
Each core computes its 4 q-heads + 1 kv head end-to-end and a partial
o_proj ([T,D]); the host sums the 4 partials per batch (TP unshard).
"""
import sys

sys.path.insert(0, "/opt/trn_rl_repo")

import numpy as np
import ml_dtypes

import concourse.bass as bass
import concourse.mybir as mybir
import concourse.tile as tile
from concourse.bass_utils import run_bass_kernel_spmd
from concourse.masks import make_identity

FP32 = mybir.dt.float32
BF16 = mybir.dt.bfloat16
AF = mybir.ActivationFunctionType
ALU = mybir.AluOpType
BF = ml_dtypes.bfloat16

B, T, D = 2, 2048, 2048
H, G, K = 16, 4, 128
HPC = H // G              # q-heads per core = 4
EPS = 1e-6
SCALE = K ** -0.5
NT = T // 128             # 16 t-tiles
NC_CHUNK = 4              # t-tiles per 512-chunk
EMPTY, FULL, PARTIAL = 0, 1, 2


def split_multiwaits(nc):
    """This container's walrus accepts one sync-wait per instruction; hoist
    extras into standalone single-wait EventSemaphore instructions."""
    n = 0
    for fn in nc.m.functions:
        for bb in fn.blocks:
            out = []
            for ins in bb.instructions:
                si = ins.sync_info
                if si is not None and si.on_wait and len(si.on_wait) > 1:
                    waits = list(si.on_wait)
                    for w in waits[:-1]:
                        n += 1
                        out.append(mybir.InstEventSemaphore(
                            name=f"{ins.name}-w{n}", engine=ins.engine,
                            ins=[], outs=[],
                            sync_info=mybir.SyncInfo(on_wait=[w], on_update=[])))
                    ins.sync_info = mybir.SyncInfo(
                        on_wait=[waits[-1]], on_update=list(si.on_update or []))
                out.append(ins)
            bb.instructions = out
    return n


def _classify(allowed):
    """allowed: [T,T] bool (t,s). Returns cls[NT,NT] in {EMPTY,FULL,PARTIAL}."""
    cls = np.zeros((NT, NT), np.int32)
    a4 = allowed.reshape(NT, 128, NT, 128)
    any_ = a4.any(axis=(1, 3))
    all_ = a4.all(axis=(1, 3))
    cls[any_ & all_] = FULL
    cls[any_ & ~all_] = PARTIAL
    return cls


def _build_schedule(segment_ids, position_ids):
    """Union schedule across batches (SPMD: one program for all cores) plus
    per-batch mask tiles for partial (i,j)."""
    allowed = []
    for b in range(B):
        pos = position_ids[b].astype(np.int64)
        seg = segment_ids[b].astype(np.int64)
        al = (pos[:, None] >= pos[None, :]) & (seg[:, None] == seg[None, :])
        allowed.append(al)
    cls_b = [_classify(al) for al in allowed]
    cls = np.maximum(cls_b[0], cls_b[1])  # EMPTY only if both empty; PARTIAL wins
    # (FULL in one batch, PARTIAL in other) -> PARTIAL with all-ones mask there
    cls[(cls_b[0] == FULL) & (cls_b[1] == PARTIAL)] = PARTIAL
    cls[(cls_b[1] == FULL) & (cls_b[0] == PARTIAL)] = PARTIAL

    mask_idx = {}
    masksT = [[], []]   # per batch: list of [128s,128t] f32 (deduped pairs)
    seen = {}
    for i in range(NT):
        for j in range(NT):
            if cls[i, j] == PARTIAL:
                subs = [allowed[b][i * 128:(i + 1) * 128, j * 128:(j + 1) * 128].T
                        for b in range(B)]
                key = subs[0].tobytes() + subs[1].tobytes()
                if key not in seen:
                    seen[key] = len(masksT[0])
                    for b in range(B):
                        masksT[b].append(subs[b].astype(np.float32))
                mask_idx[(i, j)] = seen[key]
    s_lists = []
    for c in range(NT // NC_CHUNK):
        js = sorted({j for i in range(c * 4, c * 4 + 4) for j in range(NT)
                     if cls[i, j] != EMPTY})
        s_lists.append(js)
    pv = {i: [j for j in range(NT) if cls[i, j] != EMPTY] for i in range(NT)}
    return cls, mask_idx, masksT, s_lists, pv


def _build_program(n_masks, cls, mask_idx, s_lists, pv):
    nc = bass.Bass()
    P = 128
    hidT = nc.declare_dram_parameter("hidT", [D, T], BF16, isOutput=False)
    wq = nc.declare_dram_parameter("wq", [D, HPC * 2 * K], BF16, isOutput=False)
    wkv = nc.declare_dram_parameter("wkv", [D, 2 * K], BF16, isOutput=False)
    wo = nc.declare_dram_parameter("wo", [HPC * K, D], BF16, isOutput=False)
    cq = nc.declare_dram_parameter("cq", [T, K], BF16, isOutput=False)
    sq_ = nc.declare_dram_parameter("sq", [T, K], BF16, isOutput=False)
    ck = nc.declare_dram_parameter("ck", [T, K], BF16, isOutput=False)
    sk = nc.declare_dram_parameter("sk", [T, K], BF16, isOutput=False)
    masks = nc.declare_dram_parameter("masks", [max(n_masks, 1), 128, 128], BF16,
                                      isOutput=False)
    out = nc.declare_dram_parameter("out", [T, D], BF16, isOutput=True)

    NDC = D // 128  # 16 d-chunks

    with tile.TileContext(nc) as tc:
        with tc.tile_pool(name="wgt", bufs=1) as wgt, \
             tc.tile_pool(name="hid", bufs=3) as hid, \
             tc.tile_pool(name="ps_big", bufs=2, space="PSUM") as ps_big, \
             tc.tile_pool(name="ps_half", bufs=4, space="PSUM") as ps_half, \
             tc.tile_pool(name="stage", bufs=3) as stage, \
             tc.tile_pool(name="ptp", bufs=4) as ptp, \
             tc.tile_pool(name="res", bufs=1) as res:

            # ---- resident loads; split/ordered so PE starts immediately ----
            ht_pre = {}
            wq_sb = res.tile([P, NDC * 1024], BF16, tag="wq")
            wkv_sb = res.tile([P, NDC * 256], BF16, tag="wkv")
            ht = hid.tile([P, NDC * 256], BF16, tag="hidT")
            nc.sync.dma_start(
                out=ht[:].rearrange("p (c j) -> p c j", c=NDC),
                in_=hidT[:].rearrange("(c p) t -> p c t", p=P)[:, :, 0:256])
            ht_pre[0] = ht
            for dc in range(NDC):
                nc.sync.dma_start(out=wq_sb[:, dc * 1024:(dc + 1) * 1024],
                                  in_=wq[dc * P:(dc + 1) * P, :])
                nc.sync.dma_start(out=wkv_sb[:, dc * 256:(dc + 1) * 256],
                                  in_=wkv[dc * P:(dc + 1) * P, :])
            tabs = {}
            for nm, t_ in (("cq", cq), ("sq", sq_), ("ck", ck), ("sk", sk)):
                tt = res.tile([P, NT * K], BF16, tag=f"tab{nm}")
                nc.sync.dma_start(out=tt[:].rearrange("p (c j) -> p c j", c=NT),
                                  in_=t_[:].rearrange("(c p) j -> p c j", p=P))
                tabs[nm] = tt
            for pr in (1, 2):
                ht = hid.tile([P, NDC * 256], BF16, tag="hidT")
                nc.gpsimd.dma_start(
                    out=ht[:].rearrange("p (c j) -> p c j", c=NDC),
                    in_=hidT[:].rearrange("(c p) t -> p c t", p=P)[
                        :, :, pr * 256:(pr + 1) * 256])
                ht_pre[pr] = ht
            mask_sb = res.tile([P, max(n_masks, 1) * 128], BF16, tag="masks")
            nc.gpsimd.dma_start(out=mask_sb[:].rearrange("p (n j) -> p n j", j=128),
                                in_=masks[:].rearrange("n p j -> p n j"))
            wo_sb = res.tile([P, HPC * D], BF16, tag="wo")
            nc.gpsimd.dma_start(out=wo_sb[:].rearrange("p (c j) -> p c j", c=HPC),
                                in_=wo[:].rearrange("(c p) j -> p c j", p=P))

            # persistent tensors
            qT = res.tile([P, HPC * T], BF16, tag="qT")          # [k, h, t]
            kT = res.tile([P, T], BF16, tag="kT")                # [k, t]
            v_sb = res.tile([P, NT * 130], BF16, tag="v")        # [s, j*130 + k], col128=1
            sg = res.tile([P, NT * 512], BF16, tag="sg")         # [t, i, h*128+k] gates
            ag = res.tile([P, NT * 512], BF16, tag="ag")         # gated attn [t, i, hk]
            epsb = res.tile([P, 1], FP32, tag="eps")

            ident = res.tile([P, P], BF16, tag="ident")
            make_identity(nc, ident)
            nc.vector.memset(epsb[:], EPS)
            nc.vector.memset(v_sb[:], 1.0)

            # =================== Phase A: projections (t-tile pairs) ==========
            for pr in range(NT // 2):
                if pr in ht_pre:
                    ht = ht_pre[pr]
                else:
                    ht = hid.tile([P, NDC * 256], BF16, tag="hidT")
                    nc.gpsimd.dma_start(
                        out=ht[:].rearrange("p (c j) -> p c j", c=NDC),
                        in_=hidT[:].rearrange("(c p) t -> p c t", p=P)[
                            :, :, pr * 256:(pr + 1) * 256])
                for tt in range(2):
                    i = pr * 2 + tt
                    qg = ps_big.tile([P, 1024], FP32, tag="big")
                    kv = ps_half.tile([P, 512], FP32, tag="half")
                    for dc in range(NDC):
                        lhsT = ht[:, dc * 256 + tt * 128: dc * 256 + tt * 128 + 128]
                        st, sp = dc == 0, dc == NDC - 1
                        nc.tensor.matmul(qg[:, 0:512], lhsT, wq_sb[:, dc * 1024:dc * 1024 + 512],
                                         start=st, stop=sp)
                        nc.tensor.matmul(qg[:, 512:1024], lhsT, wq_sb[:, dc * 1024 + 512:dc * 1024 + 1024],
                                         start=st, stop=sp)
                        nc.tensor.matmul(kv[:, 0:256], lhsT, wkv_sb[:, dc * 256:(dc + 1) * 256],
                                         start=st, stop=sp)
                    # sum of squares per head (ACT Square + accum)
                    ss = stage.tile([P, 8], FP32, tag="ss")
                    scr = stage.tile([P, 128], FP32, tag="scr")
                    for h in range(HPC):
                        nc.scalar.activation(scr[:], qg[:, h * 256:h * 256 + 128],
                                             AF.Square, accum_out=ss[:, h:h + 1])
                    nc.scalar.activation(scr[:], kv[:, 0:128], AF.Square,
                                         accum_out=ss[:, 4:5])
                    # invrms = exp(-0.5*ln(ss/128+eps))
                    ir = stage.tile([P, 8], FP32, tag="ir")
                    nc.scalar.activation(ir[:, 0:5], ss[:, 0:5], AF.Ln,
                                         scale=1.0 / K, bias=epsb[:])
                    nc.scalar.activation(ir[:, 0:5], ir[:, 0:5], AF.Exp, scale=-0.5)
                    # qn = q_raw * invrms (per head), psum->sbuf bf16
                    qn = stage.tile([P, 512], BF16, tag="qn")
                    for h in range(HPC):
                        nc.vector.tensor_scalar_mul(
                            qn[:, h * 128:(h + 1) * 128],
                            qg[:, h * 256:h * 256 + 128], ir[:, h:h + 1])
                    kn = stage.tile([P, 128], BF16, tag="kn")
                    nc.vector.tensor_scalar_mul(kn[:], kv[:, 0:128], ir[:, 4:5])
                    # gates -> sg (sigmoid applied later in one pass)
                    nc.vector.tensor_copy(
                        sg[:, i * 512:(i + 1) * 512].rearrange("p (a k) -> p a k", k=128),
                        qg[:].rearrange("p (a b k) -> p a b k", b=2, k=128)[:, :, 1, :])
                    # v evict (keeps ones at col 128 of each 130-block)
                    nc.vector.tensor_copy(v_sb[:, i * 130:i * 130 + 128], kv[:, 128:256])
                    # RoPE q (4 heads batched): qr = qn*cq + rot(qn)*sq
                    qr = stage.tile([P, 512], BF16, tag="qr")
                    tmp = stage.tile([P, 512], BF16, tag="tmp")
                    cqt = tabs["cq"][:, i * K:(i + 1) * K]
                    sqt = tabs["sq"][:, i * K:(i + 1) * K]
                    nc.vector.tensor_tensor(
                        tmp[:].rearrange("p (a k) -> p a k", k=128),
                        qn[:].rearrange("p (a k) -> p a k", k=128),
                        cqt[:, None, :].to_broadcast((P, HPC, K)), ALU.mult)
                    nc.vector.tensor_tensor(
                        qr[:].rearrange("p (a k) -> p a k", k=128)[:, :, 0:64],
                        qn[:].rearrange("p (a k) -> p a k", k=128)[:, :, 64:128],
                        sqt[:, None, 0:64].to_broadcast((P, HPC, 64)), ALU.mult)
                    nc.vector.tensor_tensor(
                        qr[:].rearrange("p (a k) -> p a k", k=128)[:, :, 64:128],
                        qn[:].rearrange("p (a k) -> p a k", k=128)[:, :, 0:64],
                        sqt[:, None, 64:128].to_broadcast((P, HPC, 64)), ALU.mult)
                    nc.vector.tensor_tensor(qr[:], qr[:], tmp[:], ALU.add)
                    # RoPE k
                    kr = stage.tile([P, 128], BF16, tag="kr")
                    tmpk = stage.tile([P, 128], BF16, tag="tmpk")
                    ckt = tabs["ck"][:, i * K:(i + 1) * K]
                    skt = tabs["sk"][:, i * K:(i + 1) * K]
                    nc.vector.tensor_tensor(tmpk[:], kn[:], ckt, ALU.mult)
                    nc.vector.tensor_tensor(kr[:, 0:64], kn[:, 64:128], skt[:, 0:64],
                                            ALU.mult)
                    nc.vector.tensor_tensor(kr[:, 64:128], kn[:, 0:64], skt[:, 64:128],
                                            ALU.mult)
                    nc.vector.tensor_tensor(kr[:], kr[:], tmpk[:], ALU.add)
                    # transposes -> qT/kT
                    for h in range(HPC):
                        nc.sync.dma_start(
                            out=qT[:, h * T + i * 128: h * T + (i + 1) * 128],
                            in_=qr[:, h * 128:(h + 1) * 128], transpose=True)
                    nc.sync.dma_start(out=kT[:, i * 128:(i + 1) * 128], in_=kr[:],
                                      transpose=True)


            # =================== Phase C/D: attention + o_proj ================
            def emit_oproj(items):
                for i, agT in items:
                    oo = stage.tile([P, D], BF16, tag="oo", bufs=4)
                    for dc in range(4):
                        ops = ps_half.tile([P, 512], FP32, tag="half")
                        for h in range(HPC):
                            nc.tensor.matmul(ops[:], agT[:, h * 128:(h + 1) * 128],
                                             wo_sb[:, h * D + dc * 512: h * D + (dc + 1) * 512],
                                             start=(h == 0), stop=(h == HPC - 1))
                        nc.vector.tensor_copy(oo[:, dc * 512:(dc + 1) * 512], ops[:])
                    nc.gpsimd.dma_start(out=out[i * 128:(i + 1) * 128, :], in_=oo[:])

            pending_oproj = None
            for c in range(NT // NC_CHUNK):
                # sigmoid(gate) for this chunk via tanh (same ACT set as exp):
                # sg = 0.5*tanh(0.5*g) + 0.5
                sgc = sg[:, c * 2048:(c + 1) * 2048]
                nc.scalar.activation(sgc, sgc, AF.Tanh, scale=0.5)
                nc.vector.tensor_scalar(sgc, sgc, 0.5, 0.5, ALU.mult, ALU.add)
                js = s_lists[c]
                groups = [js[x:x + 2] for x in range(0, len(js), 2)]
                pt_of = {}   # j -> (tile, slot)
                for h in range(HPC):
                    qslice = qT[:, h * T + c * 512: h * T + (c + 1) * 512]
                    for grp in groups:
                        ptp_ps = ps_big.tile([P, 1024], FP32, tag="big")
                        for s_i, j in enumerate(grp):
                            nc.tensor.matmul(
                                ptp_ps[:, s_i * 512:(s_i + 1) * 512],
                                kT[:, j * 128:(j + 1) * 128], qslice,
                                start=True, stop=True)
                        pt_sb = ptp.tile([P, 1024], BF16, tag="ptsb")
                        nc.scalar.activation(pt_sb[:, 0:len(grp) * 512],
                                             ptp_ps[:, 0:len(grp) * 512], AF.Exp)
                        for s_i, j in enumerate(grp):
                            pt_of[(h, j)] = (pt_sb, s_i)
                            for r in range(4):
                                i = c * 4 + r
                                if cls[i, j] == PARTIAL:
                                    m = mask_idx[(i, j)]
                                    sl = pt_sb[:, s_i * 512 + r * 128: s_i * 512 + (r + 1) * 128]
                                    nc.vector.tensor_tensor(
                                        sl, sl, mask_sb[:, m * 128:(m + 1) * 128],
                                        ALU.mult)
                    # PV + gating per t-tile for this head
                    for r in range(4):
                        i = c * 4 + r
                        jl = pv[i]
                        pvp = ps_half.tile([P, 512], FP32, tag="half")
                        for n_j, j in enumerate(jl):
                            pt_sb, s_i = pt_of[(h, j)]
                            nc.tensor.matmul(
                                pvp[:, 0:129], pt_sb[:, s_i * 512 + r * 128: s_i * 512 + (r + 1) * 128],
                                v_sb[:, j * 130:j * 130 + 129],
                                start=(n_j == 0), stop=(n_j == len(jl) - 1))
                        rd = stage.tile([P, 1], FP32, tag="rd")
                        nc.vector.reciprocal(rd[:], pvp[:, 128:129])
                        nc.vector.scalar_tensor_tensor(
                            ag[:, i * 512 + h * 128: i * 512 + (h + 1) * 128],
                            pvp[:, 0:128], rd[:],
                            sg[:, i * 512 + h * 128: i * 512 + (h + 1) * 128],
                            ALU.mult, ALU.mult)
                # transposes now; o_proj matmuls deferred one chunk so the
                # sync-queue transposes drain under the next chunk's compute
                agTs = []
                for r in range(4):
                    i = c * 4 + r
                    agT = stage.tile([P, 512], BF16, tag="agT", bufs=12)
                    for h in range(HPC):
                        nc.sync.dma_start(out=agT[:, h * 128:(h + 1) * 128],
                                          in_=ag[:, i * 512 + h * 128: i * 512 + (h + 1) * 128],
                                          transpose=True)
                    agTs.append((i, agT))
                if pending_oproj:
                    emit_oproj(pending_oproj)
                pending_oproj = agTs
            emit_oproj(pending_oproj)

    split_multiwaits(nc)
    return nc


def _install_ntff_hook():
    """Best-effort NTFF profiling hook (axon containers); harmless if absent."""
    import contextlib, ctypes, types
    if "antenv.axon_hooks" in sys.modules:
        return
    lib = ctypes.CDLL("/opt/axon/libaxon_pjrt.so")
    if not hasattr(lib, "axon_start_nrt_profile"):
        raise RuntimeError("no profile symbols")
    lib.axon_start_nrt_profile.argtypes = [ctypes.POINTER(ctypes.c_int64), ctypes.c_size_t]
    lib.axon_start_nrt_profile.restype = ctypes.c_int64
    lib.axon_stop_nrt_profile.argtypes = [ctypes.c_char_p]
    lib.axon_stop_nrt_profile.restype = ctypes.c_int64

    @contextlib.contextmanager
    def _hook(output_dir, device_ids):
        import jax
        jax.devices()
        if device_ids:
            ids = (ctypes.c_int64 * len(device_ids))(*device_ids)
            rc = lib.axon_start_nrt_profile(ids, len(device_ids))
        else:
            rc = lib.axon_start_nrt_profile(None, 0)
        if rc != 0:
            raise RuntimeError(f"axon_start_nrt_profile rc={rc}")
        try:
            yield
        finally:
            lib.axon_stop_nrt_profile(str(output_dir).encode())

    store = {"h": _hook}
    mod = types.ModuleType("antenv.axon_hooks")
    mod.get_axon_ntff_profile_hook = lambda: store.get("h")
    mod.set_axon_ntff_profile_hook = lambda h: store.__setitem__("h", h)
    import antenv
    antenv.axon_hooks = mod
    sys.modules["antenv.axon_hooks"] = mod


def kernel(hidden, cos, sin, segment_ids, position_ids, Wq, Wk, Wv, Wo,
           q_norm_w, k_norm_w):
    hidden = np.asarray(hidden, np.float32)
    cos = np.asarray(cos, np.float32)
    sin = np.asarray(sin, np.float32)
    segment_ids = np.asarray(segment_ids)
    position_ids = np.asarray(position_ids)
    Wq = np.asarray(Wq, np.float32)
    Wk = np.asarray(Wk, np.float32)
    Wv = np.asarray(Wv, np.float32)
    Wo = np.asarray(Wo, np.float32)
    q_norm_w = np.asarray(q_norm_w, np.float32)
    k_norm_w = np.asarray(k_norm_w, np.float32)

    cls, mask_idx, masksT, s_lists, pv = _build_schedule(segment_ids, position_ids)
    n_masks = len(masksT[0])

    # RoPE tables with norm weights / scale / rotate-sign folded in.
    rolled_q = np.roll(q_norm_w, -64)      # w[(k+64)%128]
    rolled_k = np.roll(k_norm_w, -64)
    sign = np.where(np.arange(K) < 64, -1.0, 1.0).astype(np.float32)
    in_maps = []
    for core in range(8):
        b, g = core // 4, core % 4
        m = dict(
            hidT=np.ascontiguousarray(hidden[b].T).astype(BF),
            wq=Wq[:, g * 1024:(g + 1) * 1024].astype(BF),
            wkv=np.concatenate([Wk[:, g * K:(g + 1) * K],
                                Wv[:, g * K:(g + 1) * K]], 1).astype(BF),
            wo=Wo[g * 512:(g + 1) * 512, :].astype(BF),
            cq=(cos[b] * q_norm_w[None, :] * SCALE).astype(BF),
            sq=(sin[b] * rolled_q[None, :] * sign[None, :] * SCALE).astype(BF),
            ck=(cos[b] * k_norm_w[None, :]).astype(BF),
            sk=(sin[b] * rolled_k[None, :] * sign[None, :]).astype(BF),
            masks=(np.stack(masksT[b], 0) if n_masks
                   else np.zeros((1, 128, 128), np.float32)).astype(BF),
        )
        in_maps.append(m)

    nc = _build_program(n_masks, cls, mask_idx, s_lists, pv)
    res = None
    try:
        _install_ntff_hook()
        res = run_bass_kernel_spmd(nc, in_maps, list(range(8)), trace=True)
    except Exception:
        res = None
    if res is None:
        res = run_bass_kernel_spmd(nc, in_maps, list(range(8)))
    out = np.zeros((B, T, D), np.float32)
    for core in range(8):
        b = core // 4
        out[b] += res.results[core]["out"].astype(np.float32)
    kernel.last_results = res
    return out


if __name__ == "__main__":
    pass
